# revision 32
# baseline (speedup 1.0000x reference)
"""Multi-head attention (b=2, l=2048, d_model=1024, h=16) on 8 trn2 NeuronCores.

Sharding: tensor-parallel over heads. Each core owns 2 heads: it computes the
QKV projections for its 128 channels (transposed layout), attention for its
heads, and a rank-128 partial of the output projection. The host sums the 8
partials and adds b_o (the tensor-parallel all-reduce, done at gather time).

v2 design (ACT-paced, PE row-tiled):
  The scalar engine (exp) is the theoretical floor: 16.8M exps/core at
  1 elem/lane/cycle @1.2GHz + 352cyc/op overhead ~= 147us. Everything else
  hides under it.
  warmup:  identity matmul burst (HAM clock ramp) + dummy exp (table preload).
  A0:      chunk-0 QKV projections only (~6us), so phase B starts early.
  B:       per (batch, 512-token q-chunk, k-tile): the two heads' scoresT
           matmuls (K=64 each) are issued back-to-back so the PE runs them
           CONCURRENTLY in row groups 0-63 / 64-127 (tile_position auto-derived
           from base_partition). One [128,1024] exp covers both heads. PV
           accumulates [65, 512] per head (ones-column computes Z). Emission is
           software-pipelined (sc(kt) -> exp(kt) -> pv(kt-1)) so ACT never
           waits. A pump queue fills PE slack with real work: remaining QKV
           chunks, V re-transposes, then normalize + output-projection units
           for finished q-chunks of both batches.
  norm:    selector matmul broadcasts Z over partitions; reciprocal_approx_fast
           (5x faster than reciprocal, plenty for softmax) + one multiply
           normalizes attnU in place, then the 2x[128,512] output projection,
           bf16 staging, DMA out.
"""
import sys
import types

import numpy as np

D_MODEL = 1024
H = 16
DH = 64
B = 2
L = 2048
BL = B * L            # 4096 tokens
NCORES = 8
NKT = D_MODEL // 128  # 8 feature tiles
TCH = 512             # phase-A token chunk
NCH = BL // TCH       # 8 chunks
QC = 512              # phase-B q chunk
NQC = L // QC         # 4 per batch
NKB = L // 128        # 16 k-tiles per batch
VSTRIDE = 2 * (DH + 1)  # per-k-tile Vaug columns: [V_h0 | 1 | V_h1 | 1]


def _register_ntff_hook():
    """Install the axon NTFF profiling hook module if the image lacks it.

    Harmless if never used; required for run_bass_kernel_spmd(trace=True)."""
    if "antenv.axon_hooks" in sys.modules:
        return
    try:
        import antenv
        mod = types.ModuleType("antenv.axon_hooks")
        holder = {}
        mod.set_axon_ntff_profile_hook = lambda h: holder.__setitem__("h", h)
        mod.get_axon_ntff_profile_hook = lambda: holder.get("h")
        sys.modules["antenv.axon_hooks"] = mod
        antenv.axon_hooks = mod
        from trn_agent_boot.trn_boot import _ntff_profile_via_ctypes
        mod.set_axon_ntff_profile_hook(
            _ntff_profile_via_ctypes("/opt/axon/libaxon_pjrt.so")
        )
    except Exception:
        pass


_NC_CACHE = {}


def _build():
    if "nc" in _NC_CACHE:
        return _NC_CACHE["nc"]
    import concourse.bacc as bacc
    import concourse.tile as tile
    import concourse.mybir as mybir

    F32 = mybir.dt.float32
    BF16 = mybir.dt.bfloat16
    AF = mybir.ActivationFunctionType
    ALU = mybir.AluOpType

    nc = bacc.Bacc("TRN2", target_bir_lowering=False, debug=False)

    xT_d = nc.dram_tensor("xT", [D_MODEL, BL], BF16, kind="ExternalInput").ap()
    wq_d = nc.dram_tensor("wq", [128, NKT * 128], BF16, kind="ExternalInput").ap()
    wk_d = nc.dram_tensor("wk", [128, NKT * 128], BF16, kind="ExternalInput").ap()
    wv_d = nc.dram_tensor("wv", [128, NKT * 128], BF16, kind="ExternalInput").ap()
    bq_d = nc.dram_tensor("bq", [128, 1], F32, kind="ExternalInput").ap()
    bk_d = nc.dram_tensor("bk", [128, 1], F32, kind="ExternalInput").ap()
    bv_d = nc.dram_tensor("bv", [128, 1], F32, kind="ExternalInput").ap()
    wo_d = nc.dram_tensor("wo", [128, D_MODEL], BF16, kind="ExternalInput").ap()
    id_d = nc.dram_tensor("ident", [128, 128], BF16, kind="ExternalInput").ap()
    out_d = nc.dram_tensor("out", [BL, D_MODEL], BF16, kind="ExternalOutput").ap()

    with tile.TileContext(nc) as tc:
        with (
            tc.tile_pool(name="weights", bufs=1) as wpool,
            tc.tile_pool(name="persist", bufs=1) as ppool,
        ):
            id_t = wpool.tile([128, 128], BF16, tag="ident")
            nc.gpsimd.dma_start(id_t[:], id_d)
            wq_t = wpool.tile([128, NKT * 128], BF16, tag="wq")
            wk_t = wpool.tile([128, NKT * 128], BF16, tag="wk")
            wv_t = wpool.tile([128, NKT * 128], BF16, tag="wv")
            bq_t = wpool.tile([128, 1], F32, tag="bq")
            bk_t = wpool.tile([128, 1], F32, tag="bk")
            bv_t = wpool.tile([128, 1], F32, tag="bv")
            wo_t = wpool.tile([128, D_MODEL], BF16, tag="wo")
            for t, d in ((wq_t, wq_d), (wk_t, wk_d), (wv_t, wv_d),
                         (bq_t, bq_d), (bk_t, bk_d), (bv_t, bv_d),
                         (wo_t, wo_d)):
                nc.gpsimd.dma_start(t[:], d)

            QT = ppool.tile([128, BL], BF16, tag="QT")
            KT = ppool.tile([128, BL], BF16, tag="KT")
            VT = ppool.tile([128, BL], BF16, tag="VT")
            Vaug = ppool.tile([128, (BL // 128) * VSTRIDE], BF16, tag="Vaug")
            attnU = [ppool.tile([128, L], BF16, tag=f"attnU{b}",
                                name=f"attnU{b}") for b in range(B)]
            # softmax denominators Z: h0 at partition 0, h1 at
            # partition 32 (engine writes need 32-aligned base partitions)
            zb = [ppool.tile([33, L], BF16, tag=f"zb{b}",
                             name=f"zb{b}") for b in range(B)]
            # head-half selector: rows 0 / 32 pick head halves, rest zero
            sel_t = ppool.tile([33, 128], BF16, tag="sel")
            scr = ppool.tile([1, 32], F32, tag="scr")

            # packed pair of bf16 1.0s viewed as f32
            ONE2 = float(np.frombuffer(
                np.uint32(0x3F803F80).tobytes(), dtype=np.float32)[0])
            nc.vector.memset(Vaug[:].bitcast(F32), ONE2)
            nc.vector.memset(sel_t[:].bitcast(F32), 0.0)
            nc.vector.memset(sel_t[:].bitcast(F32)[0:1, 0:32], ONE2)
            nc.vector.memset(sel_t[:].bitcast(F32)[32:33, 32:64], ONE2)
            for b in range(B):
                nc.vector.memset(zb[b][:].bitcast(F32), ONE2)

            with (
                tc.tile_pool(name="xin", bufs=2) as xpool,
                tc.tile_pool(name="scaleP", bufs=2) as spool,
                tc.tile_pool(name="expP", bufs=3) as epool,
                tc.tile_pool(name="a65P", bufs=2) as apool,
                tc.tile_pool(name="oout", bufs=3) as opool,
                tc.tile_pool(name="psX", bufs=2, space="PSUM") as psX,
                tc.tile_pool(name="psS", bufs=2, space="PSUM") as psS,
                tc.tile_pool(name="psPV", bufs=1, space="PSUM") as psPV,
            ):
                # ---- warmup: preload exp table + lift clock gate ----
                # wue is memset (no DMA dependency) so the warmup burst and
                # the ACT table load start immediately, overlapping the
                # weight/x DMAs.
                wue = ppool.tile([128, 128], BF16, tag="wue")
                nc.vector.memset(wue[:].bitcast(F32), ONE2)
                nc.scalar.activation(scr[:], wue[0:1, 0:64].bitcast(F32),
                                     AF.Exp)
                wu = psX.tile([128, 512], F32, tag="x")
                for i in range(40):
                    nc.tensor.matmul(wu[:, 0:128], wue[:], wue[:],
                                     start=(i == 0), stop=(i == 39))

                chunk_xt = {}

                def emit_chunk_dma(c):
                    xt = xpool.tile([128, NKT, TCH], BF16, tag="xchunk",
                                    name=f"xt{c}")
                    sl = slice(c * TCH, (c + 1) * TCH)
                    for kt in range(NKT):
                        nc.sync.dma_start(
                            xt[:, kt, :], xT_d[kt * 128:(kt + 1) * 128, sl]
                        )
                    chunk_xt[c] = xt

                proj_ps = {}

                def emit_proj_half(c, key, w_t, b_t, dst, half):
                    # one QKV projection = 8 accumulating matmuls, split in
                    # two 4-matmul halves sized to phase B's per-step PE
                    # slack. The two halves MUST be adjacent pump items:
                    # with psX bufs=2, one foreign allocation between them
                    # is safe, two would recycle the held buffer.
                    if half == 0:
                        ps = psX.tile([128, TCH], F32, tag="x",
                                      name=f"pj{key}{c}")
                        proj_ps[(key, c)] = ps
                    else:
                        ps = proj_ps.pop((key, c))
                    xt = chunk_xt[c]
                    for kt in range(half * 4, half * 4 + 4):
                        nc.tensor.matmul(
                            ps[:], w_t[:, kt * 128:(kt + 1) * 128],
                            xt[:, kt, :],
                            start=(kt == 0), stop=(kt == NKT - 1),
                        )
                    if half == 1:
                        sl = slice(c * TCH, (c + 1) * TCH)
                        nc.vector.tensor_scalar_add(dst[:, sl], ps[:],
                                                    b_t[:, 0:1])

                def emit_proj(c, key, w_t, b_t, dst):
                    emit_proj_half(c, key, w_t, b_t, dst, 0)
                    emit_proj_half(c, key, w_t, b_t, dst, 1)

                def emit_tr(c):
                    # natural-layout V (with ones cols) for this chunk's tiles
                    for g in range(c * (TCH // 128), (c + 1) * (TCH // 128)):
                        ps = psX.tile([128, 512], F32, tag="x", name="trps")
                        tp = ps.bitcast(BF16)
                        nc.tensor.transpose(
                            tp[:, 0:128], VT[:, g * 128:(g + 1) * 128], id_t[:]
                        )
                        base = g * VSTRIDE
                        nc.vector.tensor_copy(
                            Vaug[:, base:base + DH], tp[:, 0:DH]
                        )
                        nc.vector.tensor_copy(
                            Vaug[:, base + DH + 1:base + 2 * DH + 1],
                            tp[:, DH:2 * DH],
                        )

                def emit_norm_qc(b, qc):
                    # normalize 512 tokens: selector matmul broadcasts Z over
                    # the channel partitions, fast reciprocal in that layout,
                    # one multiply normalizes attnU in place
                    ps = psX.tile([128, 512], F32, tag="x", name="scaleps")
                    jsl = slice(qc * QC, (qc + 1) * QC)
                    nc.tensor.matmul(ps[:], sel_t[:], zb[b][:, jsl],
                                     start=True, stop=True)
                    ss = spool.tile([128, 512], F32, tag="ss", name="sstile")
                    nc.vector.reciprocal_approx_fast(ss[:], ps[:])
                    nc.vector.tensor_tensor(
                        attnU[b][:, jsl], attnU[b][:, jsl], ss[:],
                        op=ALU.mult,
                    )

                def emit_cu_rc(b, rc, tail=False):
                    # output projection for 128 tokens: out[tok, :] =
                    # attnN.T @ Wo, staged to bf16 and DMA'd out.
                    # In-B: matmul writes bf16 PSUM so the staging copy runs
                    # at the DVE's 2x 16-bit rate. Tail (post-B): the free
                    # scores banks hold both halves, one wide copy, ACT/DVE
                    # alternating.
                    jsl = slice(rc * 128, (rc + 1) * 128)
                    st = opool.tile([128, 1024], BF16, tag="cu", name="cust")
                    if tail:
                        ps = psS.tile([128, 2 * QC], F32, tag="sc",
                                      name="cutps")
                        for oc in range(2):
                            osl = slice(oc * 512, (oc + 1) * 512)
                            nc.tensor.matmul(ps[:, osl], attnU[b][:, jsl],
                                             wo_t[:, osl],
                                             start=True, stop=True)
                        if rc % 2 == 0:
                            nc.scalar.activation(st[:], ps[:], AF.Copy)
                        else:
                            nc.vector.tensor_copy(st[:], ps[:])
                    else:
                        for oc in range(2):
                            osl = slice(oc * 512, (oc + 1) * 512)
                            ps = psX.tile([128, 512], F32, tag="x",
                                          name="cups")
                            nc.tensor.matmul(ps[:], attnU[b][:, jsl],
                                             wo_t[:, osl],
                                             start=True, stop=True)
                            nc.vector.tensor_copy(st[:, osl], ps[:])
                    grow = b * 16 + rc
                    nc.sync.dma_start(
                        out_d[grow * 128:(grow + 1) * 128, :], st[:]
                    )

                # ---- A0: chunk 0 only, so phase B starts early ----
                emit_chunk_dma(0)
                emit_chunk_dma(1)
                emit_proj(0, "k", wk_t, bk_t, KT)
                emit_proj(0, "q", wq_t, bq_t, QT)
                emit_proj(0, "v", wv_t, bv_t, VT)
                emit_tr(0)

                # ---- side-work queues for phase B's PE slack ----
                # NOTE: chunk c's Q-projection must be EMITTED before chunk
                # c+2's DMA (xpool bufs=2 reuses its buffer) so the WAR
                # dependency is tracked; keeping each chunk's group together
                # guarantees this.
                a1 = []

                def add_chunk(c):
                    a1.append(lambda c=c: None if c + 1 >= NCH
                              else emit_chunk_dma(c + 1))
                    for key, w_t, b_t, dst in (("k", wk_t, bk_t, KT),
                                               ("v", wv_t, bv_t, VT)):
                        for h in range(2):
                            a1.append(lambda c=c, key=key, w_t=w_t, b_t=b_t,
                                      dst=dst, h=h:
                                      emit_proj_half(c, key, w_t, b_t,
                                                     dst, h))
                    a1.append(lambda c=c: emit_tr(c))
                    for h in range(2):
                        a1.append(lambda c=c, h=h:
                                  emit_proj_half(c, "q", wq_t, bq_t, QT, h))

                for c in range(1, NCH):
                    add_chunk(c)

                pc = []

                # Side-work runs in a high-value priority band: the Tile
                # scheduler then only slots it where the critical path
                # (scores -> exp -> PV) leaves the engines idle.
                import contextlib
                SIDE_BASE = 5_000_000
                side_ctr = [0]

                @contextlib.contextmanager
                def sidep():
                    save = tc.cur_priority
                    tc.cur_priority = SIDE_BASE + side_ctr[0]
                    try:
                        yield
                    finally:
                        side_ctr[0] = tc.cur_priority - SIDE_BASE
                        tc.cur_priority = save

                def run_pc(item, tail=False):
                    kind, b, i = item
                    if kind == "norm":
                        emit_norm_qc(b, i)
                    else:
                        emit_cu_rc(b, i, tail=tail)

                def pump(allow_pc=True):
                    # 4 pops: the head-major kt sweep consumes K-chunk c by
                    # step 2c, so chunk projections must emit 2x faster than
                    # the kt-major order needed
                    with sidep():
                        for _ in range(4):
                            if a1:
                                a1.pop(0)()
                            elif pc and allow_pc:
                                run_pc(pc.pop(0))

                # ---- B: attention, ACT-paced ----
                # Each step is ONE head x TWO k-tiles (same real PE cost as
                # a dual-head row-tiled pair, but the scheduler's cost model
                # doesn't know row-group concurrency and would double-count
                # the pair, hiding all PE slack from the side-work band).
                pending = [None]
                for b in range(B):
                    for qc in range(NQC):
                        q0 = b * L + qc * QC
                        lqsl = slice(qc * QC, (qc + 1) * QC)
                        for h in range(2):
                            hs = slice(h * 64, (h + 1) * 64)
                            pv = psPV.tile([65, QC], F32, tag=f"pv{h}",
                                           name=f"pv{h}")
                            for kp in range(NKB // 2):
                                ka = b * L + 2 * kp * 128
                                sc = psS.tile([128, 2 * QC], F32, tag="sc")
                                nc.tensor.matmul(
                                    sc[:, 0:QC], KT[hs, ka:ka + 128],
                                    QT[hs, q0:q0 + QC],
                                    start=True, stop=True)
                                nc.tensor.matmul(
                                    sc[:, QC:2 * QC],
                                    KT[hs, ka + 128:ka + 256],
                                    QT[hs, q0:q0 + QC],
                                    start=True, stop=True)
                                ex = epool.tile([128, 2 * QC], BF16,
                                                tag="ex")
                                nc.scalar.activation(ex[:], sc[:], AF.Exp,
                                                     scale=0.125)
                                if pending[0] is not None:
                                    pending[0]()
                                g = b * NKB + 2 * kp

                                def mk_pv(g=g, kp=kp, h=h, ex=ex, pv=pv):
                                    va = g * VSTRIDE + h * (DH + 1)
                                    vb = (g + 1) * VSTRIDE + h * (DH + 1)

                                    def f():
                                        nc.tensor.matmul(
                                            pv[:], Vaug[:, va:va + DH + 1],
                                            ex[:, 0:QC],
                                            start=(kp == 0), stop=False,
                                        )
                                        nc.tensor.matmul(
                                            pv[:], Vaug[:, vb:vb + DH + 1],
                                            ex[:, QC:2 * QC],
                                            start=False,
                                            stop=(kp == NKB // 2 - 1),
                                        )
                                    return f
                                pending[0] = mk_pv()
                                # late-b1 norm/oproj units go to the post-B
                                # tail: pumping them here makes the scheduler
                                # cram them into the last qc-boundary bubble
                                pump(allow_pc=not (b == 1 and qc >= 2))
                            pending[0]()
                            pending[0] = None
                            a65 = apool.tile([65, QC], BF16, tag="a65")
                            nc.vector.tensor_copy(a65[:], pv[:])
                            nc.vector.tensor_copy(
                                attnU[b][h * 64:(h + 1) * 64, lqsl],
                                a65[0:64, :],
                            )
                            nc.vector.tensor_copy(
                                zb[b][32 * h:32 * h + 1, lqsl],
                                a65[64:65, :],
                            )
                        pc.append(("norm", b, qc))
                        for rc in range(qc * 4, qc * 4 + 4):
                            pc.append(("cu", b, rc))

                # drain whatever the slots didn't absorb (post-B: tail mode)
                with sidep():
                    while a1:
                        a1.pop(0)()
                    while pc:
                        run_pc(pc.pop(0), tail=True)

    nc.compile()
    _NC_CACHE["nc"] = nc
    return nc


def _shard_inputs(x, W_qkv, b_qkv, W_o):
    import ml_dtypes
    BF = ml_dtypes.bfloat16
    xT = np.ascontiguousarray(
        x.reshape(BL, D_MODEL).T.astype(BF)
    )
    ident = np.eye(128, dtype=BF)

    def lhsT_layout(w):
        # [D_MODEL, 128] -> [128, NKT*128] with [p, kt*128+ch] = w[kt*128+p, ch]
        return np.ascontiguousarray(
            w.reshape(NKT, 128, 128).transpose(1, 0, 2)
            .reshape(128, NKT * 128).astype(BF)
        )

    in_maps = []
    for c in range(NCORES):
        cs = slice(c * 128, (c + 1) * 128)
        wq = W_qkv[:, cs]
        wk = W_qkv[:, D_MODEL:][:, cs]
        wv = W_qkv[:, 2 * D_MODEL:][:, cs]
        in_maps.append({
            "xT": xT,
            "wq": lhsT_layout(wq), "wk": lhsT_layout(wk),
            "wv": lhsT_layout(wv),
            "bq": np.ascontiguousarray(
                b_qkv[cs], dtype=np.float32).reshape(128, 1),
            "bk": np.ascontiguousarray(
                b_qkv[D_MODEL:][cs], dtype=np.float32).reshape(128, 1),
            "bv": np.ascontiguousarray(
                b_qkv[2 * D_MODEL:][cs], dtype=np.float32).reshape(128, 1),
            "wo": np.ascontiguousarray(W_o[cs, :].astype(BF)),
            "ident": ident,
        })
    return in_maps


def _run(inputs, trace=False, tmpdir=None):
    from concourse.bass_utils import run_bass_kernel_spmd

    _register_ntff_hook()
    nc = _build()
    in_maps = _shard_inputs(
        np.asarray(inputs["x"], dtype=np.float32),
        np.asarray(inputs["W_qkv"], dtype=np.float32),
        np.asarray(inputs["b_qkv"], dtype=np.float32),
        np.asarray(inputs["W_o"], dtype=np.float32),
    )
    res = run_bass_kernel_spmd(nc, in_maps, core_ids=list(range(NCORES)),
                               trace=trace, tmpdir=tmpdir)
    partial = np.zeros((BL, D_MODEL), dtype=np.float64)
    for c in range(NCORES):
        partial += res.results[c]["out"].astype(np.float64)
    out = (partial + np.asarray(inputs["b_o"], dtype=np.float64)).astype(np.float32)
    return out.reshape(B, L, D_MODEL), res


def kernel(**inputs) -> np.ndarray:
    out, _ = _run(inputs, trace=False)
    return out


# revision 38
# speedup vs baseline: 1.0540x; 1.0540x over previous
"""Multi-head attention (b=2, l=2048, d_model=1024, h=16) on 8 trn2 NeuronCores.

Sharding: tensor-parallel over heads. Each core owns 2 heads: it computes the
QKV projections for its 128 channels (transposed layout), attention for its
heads, and a rank-128 partial of the output projection. The host sums the 8
partials and adds b_o (the tensor-parallel all-reduce, done at gather time).

v2 design (ACT-paced, PE row-tiled):
  The scalar engine (exp) is the theoretical floor: 16.8M exps/core at
  1 elem/lane/cycle @1.2GHz + 352cyc/op overhead ~= 147us. Everything else
  hides under it.
  warmup:  identity matmul burst (HAM clock ramp) + dummy exp (table preload).
  A0:      chunk-0 QKV projections only (~6us), so phase B starts early.
  B:       per (batch, 512-token q-chunk, k-tile): the two heads' scoresT
           matmuls (K=64 each) are issued back-to-back so the PE runs them
           CONCURRENTLY in row groups 0-63 / 64-127 (tile_position auto-derived
           from base_partition). One [128,1024] exp covers both heads. PV
           accumulates [65, 512] per head (ones-column computes Z). Emission is
           software-pipelined (sc(kt) -> exp(kt) -> pv(kt-1)) so ACT never
           waits. A pump queue fills PE slack with real work: remaining QKV
           chunks, V re-transposes, then normalize + output-projection units
           for finished q-chunks of both batches.
  norm:    selector matmul broadcasts Z over partitions; reciprocal_approx_fast
           (5x faster than reciprocal, plenty for softmax) + one multiply
           normalizes attnU in place, then the 2x[128,512] output projection,
           bf16 staging, DMA out.
"""
import sys
import types

import numpy as np

D_MODEL = 1024
H = 16
DH = 64
B = 2
L = 2048
BL = B * L            # 4096 tokens
NCORES = 8
NKT = D_MODEL // 128  # 8 feature tiles
TCH = 512             # phase-A token chunk
NCH = BL // TCH       # 8 chunks
QC = 512              # phase-B q chunk
NQC = L // QC         # 4 per batch
NKB = L // 128        # 16 k-tiles per batch
VSTRIDE = 2 * (DH + 1)  # per-k-tile Vaug columns: [V_h0 | 1 | V_h1 | 1]


def _register_ntff_hook():
    """Install the axon NTFF profiling hook module if the image lacks it.

    Harmless if never used; required for run_bass_kernel_spmd(trace=True)."""
    if "antenv.axon_hooks" in sys.modules:
        return
    try:
        import antenv
        mod = types.ModuleType("antenv.axon_hooks")
        holder = {}
        mod.set_axon_ntff_profile_hook = lambda h: holder.__setitem__("h", h)
        mod.get_axon_ntff_profile_hook = lambda: holder.get("h")
        sys.modules["antenv.axon_hooks"] = mod
        antenv.axon_hooks = mod
        from trn_agent_boot.trn_boot import _ntff_profile_via_ctypes
        mod.set_axon_ntff_profile_hook(
            _ntff_profile_via_ctypes("/opt/axon/libaxon_pjrt.so")
        )
    except Exception:
        pass


_NC_CACHE = {}


def _build():
    if "nc" in _NC_CACHE:
        return _NC_CACHE["nc"]
    import concourse.bacc as bacc
    import concourse.tile as tile
    import concourse.mybir as mybir

    F32 = mybir.dt.float32
    BF16 = mybir.dt.bfloat16
    AF = mybir.ActivationFunctionType
    ALU = mybir.AluOpType

    nc = bacc.Bacc("TRN2", target_bir_lowering=False, debug=False)

    xT_d = nc.dram_tensor("xT", [D_MODEL, BL], BF16, kind="ExternalInput").ap()
    wq_d = nc.dram_tensor("wq", [128, NKT * 128], BF16, kind="ExternalInput").ap()
    wk_d = nc.dram_tensor("wk", [128, NKT * 128], BF16, kind="ExternalInput").ap()
    wv_d = nc.dram_tensor("wv", [128, NKT * 128], BF16, kind="ExternalInput").ap()
    bq_d = nc.dram_tensor("bq", [128, 1], F32, kind="ExternalInput").ap()
    bk_d = nc.dram_tensor("bk", [128, 1], F32, kind="ExternalInput").ap()
    bv_d = nc.dram_tensor("bv", [128, 1], F32, kind="ExternalInput").ap()
    wo_d = nc.dram_tensor("wo", [128, D_MODEL], BF16, kind="ExternalInput").ap()
    id_d = nc.dram_tensor("ident", [128, 128], BF16, kind="ExternalInput").ap()
    out_d = nc.dram_tensor("out", [BL, D_MODEL], BF16, kind="ExternalOutput").ap()

    with tile.TileContext(nc) as tc:
        with (
            tc.tile_pool(name="weights", bufs=1) as wpool,
            tc.tile_pool(name="persist", bufs=1) as ppool,
        ):
            id_t = wpool.tile([128, 128], BF16, tag="ident")
            nc.gpsimd.dma_start(id_t[:], id_d)
            wq_t = wpool.tile([128, NKT * 128], BF16, tag="wq")
            wk_t = wpool.tile([128, NKT * 128], BF16, tag="wk")
            wv_t = wpool.tile([128, NKT * 128], BF16, tag="wv")
            bq_t = wpool.tile([128, 1], F32, tag="bq")
            bk_t = wpool.tile([128, 1], F32, tag="bk")
            bv_t = wpool.tile([128, 1], F32, tag="bv")
            wo_t = wpool.tile([128, D_MODEL], BF16, tag="wo")
            for t, d in ((wq_t, wq_d), (wk_t, wk_d), (wv_t, wv_d),
                         (bq_t, bq_d), (bk_t, bk_d), (bv_t, bv_d),
                         (wo_t, wo_d)):
                nc.gpsimd.dma_start(t[:], d)

            QT = ppool.tile([128, BL], BF16, tag="QT")
            KT = ppool.tile([128, BL], BF16, tag="KT")
            VT = ppool.tile([128, BL], BF16, tag="VT")
            Vaug = ppool.tile([128, (BL // 128) * VSTRIDE], BF16, tag="Vaug")
            attnU = [ppool.tile([128, L], BF16, tag=f"attnU{b}",
                                name=f"attnU{b}") for b in range(B)]
            # softmax denominators Z: h0 at partition 0, h1 at
            # partition 32 (engine writes need 32-aligned base partitions)
            zb = [ppool.tile([33, L], BF16, tag=f"zb{b}",
                             name=f"zb{b}") for b in range(B)]
            # head-half selector: rows 0 / 32 pick head halves, rest zero
            sel_t = ppool.tile([33, 128], BF16, tag="sel")
            scr = ppool.tile([1, 32], F32, tag="scr")

            # packed pair of bf16 1.0s viewed as f32
            ONE2 = float(np.frombuffer(
                np.uint32(0x3F803F80).tobytes(), dtype=np.float32)[0])
            nc.vector.memset(Vaug[:].bitcast(F32), ONE2)
            nc.vector.memset(sel_t[:].bitcast(F32), 0.0)
            nc.vector.memset(sel_t[:].bitcast(F32)[0:1, 0:32], ONE2)
            nc.vector.memset(sel_t[:].bitcast(F32)[32:33, 32:64], ONE2)
            for b in range(B):
                nc.vector.memset(zb[b][:].bitcast(F32), ONE2)

            with (
                tc.tile_pool(name="xin", bufs=3) as xpool,
                tc.tile_pool(name="scaleP", bufs=2) as spool,
                tc.tile_pool(name="expP", bufs=3) as epool,
                tc.tile_pool(name="a65P", bufs=2) as apool,
                tc.tile_pool(name="oout", bufs=3) as opool,
                tc.tile_pool(name="psX", bufs=2, space="PSUM") as psX,
                tc.tile_pool(name="psS", bufs=2, space="PSUM") as psS,
                tc.tile_pool(name="psPV", bufs=1, space="PSUM") as psPV,
            ):
                # ---- warmup: preload exp table + lift clock gate ----
                # wue is memset (no DMA dependency) so the warmup burst and
                # the ACT table load start immediately, overlapping the
                # weight/x DMAs.
                wue = ppool.tile([128, 128], BF16, tag="wue")
                nc.vector.memset(wue[:].bitcast(F32), ONE2)
                nc.scalar.activation(scr[:], wue[0:1, 0:64].bitcast(F32),
                                     AF.Exp)
                wu = psX.tile([128, 512], F32, tag="x")
                for i in range(40):
                    nc.tensor.matmul(wu[:, 0:128], wue[:], wue[:],
                                     start=(i == 0), stop=(i == 39))

                chunk_xt = {}

                def emit_chunk_dma(c):
                    xt = xpool.tile([128, NKT, TCH], BF16, tag="xchunk",
                                    name=f"xt{c}")
                    sl = slice(c * TCH, (c + 1) * TCH)
                    for kt in range(NKT):
                        nc.sync.dma_start(
                            xt[:, kt, :], xT_d[kt * 128:(kt + 1) * 128, sl]
                        )
                    chunk_xt[c] = xt

                proj_ps = {}

                def emit_proj_half(c, key, w_t, b_t, dst, half):
                    # one QKV projection = 8 accumulating matmuls, split in
                    # two 4-matmul halves sized to phase B's per-step PE
                    # slack. The two halves MUST be adjacent pump items:
                    # with psX bufs=2, one foreign allocation between them
                    # is safe, two would recycle the held buffer.
                    if half == 0:
                        ps = psX.tile([128, TCH], F32, tag="x",
                                      name=f"pj{key}{c}")
                        proj_ps[(key, c)] = ps
                    else:
                        ps = proj_ps.pop((key, c))
                    xt = chunk_xt[c]
                    for kt in range(half * 4, half * 4 + 4):
                        nc.tensor.matmul(
                            ps[:], w_t[:, kt * 128:(kt + 1) * 128],
                            xt[:, kt, :],
                            start=(kt == 0), stop=(kt == NKT - 1),
                        )
                    if half == 1:
                        sl = slice(c * TCH, (c + 1) * TCH)
                        nc.vector.tensor_scalar_add(dst[:, sl], ps[:],
                                                    b_t[:, 0:1])

                def emit_proj(c, key, w_t, b_t, dst):
                    emit_proj_half(c, key, w_t, b_t, dst, 0)
                    emit_proj_half(c, key, w_t, b_t, dst, 1)

                def emit_tr(c):
                    # natural-layout V (with ones cols) for this chunk's tiles
                    for g in range(c * (TCH // 128), (c + 1) * (TCH // 128)):
                        ps = psX.tile([128, 512], F32, tag="x", name="trps")
                        tp = ps.bitcast(BF16)
                        nc.tensor.transpose(
                            tp[:, 0:128], VT[:, g * 128:(g + 1) * 128], id_t[:]
                        )
                        base = g * VSTRIDE
                        nc.vector.tensor_copy(
                            Vaug[:, base:base + DH], tp[:, 0:DH]
                        )
                        nc.vector.tensor_copy(
                            Vaug[:, base + DH + 1:base + 2 * DH + 1],
                            tp[:, DH:2 * DH],
                        )

                def emit_norm_qc(b, qc):
                    # normalize 512 tokens: selector matmul broadcasts Z over
                    # the channel partitions, fast reciprocal in that layout,
                    # one multiply normalizes attnU in place
                    ps = psX.tile([128, 512], F32, tag="x", name="scaleps")
                    jsl = slice(qc * QC, (qc + 1) * QC)
                    nc.tensor.matmul(ps[:], sel_t[:], zb[b][:, jsl],
                                     start=True, stop=True)
                    ss = spool.tile([128, 512], F32, tag="ss", name="sstile")
                    nc.vector.reciprocal_approx_fast(ss[:], ps[:])
                    nc.vector.tensor_tensor(
                        attnU[b][:, jsl], attnU[b][:, jsl], ss[:],
                        op=ALU.mult,
                    )

                def emit_cu_rc(b, rc, tail=False):
                    # output projection for 128 tokens: out[tok, :] =
                    # attnN.T @ Wo, staged to bf16 and DMA'd out.
                    # In-B: matmul writes bf16 PSUM so the staging copy runs
                    # at the DVE's 2x 16-bit rate. Tail (post-B): the free
                    # scores banks hold both halves, one wide copy, ACT/DVE
                    # alternating.
                    jsl = slice(rc * 128, (rc + 1) * 128)
                    st = opool.tile([128, 1024], BF16, tag="cu", name="cust")
                    if tail:
                        ps = psS.tile([128, 2 * QC], F32, tag="sc",
                                      name="cutps")
                        for oc in range(2):
                            osl = slice(oc * 512, (oc + 1) * 512)
                            nc.tensor.matmul(ps[:, osl], attnU[b][:, jsl],
                                             wo_t[:, osl],
                                             start=True, stop=True)
                        if rc % 2 == 0:
                            nc.scalar.activation(st[:], ps[:], AF.Copy)
                        else:
                            nc.vector.tensor_copy(st[:], ps[:])
                    else:
                        for oc in range(2):
                            osl = slice(oc * 512, (oc + 1) * 512)
                            ps = psX.tile([128, 512], F32, tag="x",
                                          name="cups")
                            nc.tensor.matmul(ps[:], attnU[b][:, jsl],
                                             wo_t[:, osl],
                                             start=True, stop=True)
                            nc.vector.tensor_copy(st[:, osl], ps[:])
                    grow = b * 16 + rc
                    nc.sync.dma_start(
                        out_d[grow * 128:(grow + 1) * 128, :], st[:]
                    )

                # ---- A0: chunk 0 only, so phase B starts early ----
                # Q for all of b0 is emitted here in the critical band: each
                # Q-proj becomes ready as its chunk DMA lands and the
                # scheduler runs it at once (low priority number), instead of
                # deferring it to a RAW-forced burst at a qc boundary.
                # xpool bufs=3 makes dma2's buffer reuse safe (chunk-0
                # readers are all emitted in A0).
                emit_chunk_dma(0)
                emit_chunk_dma(1)
                emit_proj(0, "k", wk_t, bk_t, KT)
                emit_proj(0, "q", wq_t, bq_t, QT)
                emit_proj(0, "v", wv_t, bv_t, VT)
                emit_tr(0)
                emit_proj(1, "q", wq_t, bq_t, QT)
                emit_chunk_dma(2)
                emit_proj(2, "q", wq_t, bq_t, QT)

                # ---- side-work queues for phase B's PE slack ----
                # NOTE: chunk c's Q-projection must be EMITTED before chunk
                # c+2's DMA (xpool bufs=2 reuses its buffer) so the WAR
                # dependency is tracked; keeping each chunk's group together
                # guarantees this.
                a1 = []

                def add_kv(c):
                    for key, w_t, b_t, dst in (("k", wk_t, bk_t, KT),
                                               ("v", wv_t, bv_t, VT)):
                        for h in range(2):
                            a1.append(lambda c=c, key=key, w_t=w_t, b_t=b_t,
                                      dst=dst, h=h:
                                      emit_proj_half(c, key, w_t, b_t,
                                                     dst, h))
                    a1.append(lambda c=c: emit_tr(c))

                def add_q(c):
                    for h in range(2):
                        a1.append(lambda c=c, h=h:
                                  emit_proj_half(c, "q", wq_t, bq_t, QT, h))

                def add_dma(c):
                    a1.append(lambda c=c: emit_chunk_dma(c))

                # dma(c) reuses buffer c%3: it must follow chunk c-3's
                # reader emissions (K/V/tr in a1, Q in A0 for b0 chunks)
                add_dma(3)
                add_q(3)
                add_kv(1)
                add_kv(2)
                add_dma(4)
                add_kv(3)
                add_dma(5)
                add_kv(4)
                add_q(4)
                add_dma(6)
                add_kv(5)
                add_q(5)
                add_dma(7)
                add_kv(6)
                add_q(6)
                add_kv(7)
                add_q(7)

                pc = []

                # Side-work runs in a high-value priority band: the Tile
                # scheduler then only slots it where the critical path
                # (scores -> exp -> PV) leaves the engines idle.
                import contextlib
                SIDE_BASE = 5_000_000
                side_ctr = [0]

                @contextlib.contextmanager
                def sidep():
                    save = tc.cur_priority
                    tc.cur_priority = SIDE_BASE + side_ctr[0]
                    try:
                        yield
                    finally:
                        side_ctr[0] = tc.cur_priority - SIDE_BASE
                        tc.cur_priority = save

                def run_pc(item, tail=False):
                    kind, b, i = item
                    if kind == "norm":
                        emit_norm_qc(b, i)
                    else:
                        emit_cu_rc(b, i, tail=tail)

                def pump(allow_pc=True):
                    with sidep():
                        for _ in range(3):
                            if a1:
                                a1.pop(0)()
                            elif pc and allow_pc:
                                run_pc(pc.pop(0))

                # ---- B: attention, ACT-paced ----
                pending = [None]
                for b in range(B):
                    for qc in range(NQC):
                        q0 = b * L + qc * QC
                        lqsl = slice(qc * QC, (qc + 1) * QC)
                        pv0 = psPV.tile([65, QC], F32, tag="pv0")
                        pv1 = psPV.tile([65, QC], F32, tag="pv1")
                        for kt in range(NKB):
                            ksl = slice(b * L + kt * 128,
                                        b * L + (kt + 1) * 128)
                            sc = psS.tile([128, 2 * QC], F32, tag="sc")
                            # two heads issued adjacently -> concurrent
                            # row-group matmuls (rows 0-63 / 64-127)
                            nc.tensor.matmul(sc[:, 0:QC], KT[0:64, ksl],
                                             QT[0:64, q0:q0 + QC],
                                             start=True, stop=True)
                            nc.tensor.matmul(sc[:, QC:2 * QC],
                                             KT[64:128, ksl],
                                             QT[64:128, q0:q0 + QC],
                                             start=True, stop=True)
                            ex = epool.tile([128, 2 * QC], BF16, tag="ex")
                            nc.scalar.activation(ex[:], sc[:], AF.Exp,
                                                 scale=0.125)
                            if pending[0] is not None:
                                pending[0]()
                            g = b * NKB + kt

                            def mk_pv(g=g, kt=kt, ex=ex, pv0=pv0, pv1=pv1):
                                vb = g * VSTRIDE

                                def f():
                                    nc.tensor.matmul(
                                        pv0[:], Vaug[:, vb:vb + DH + 1],
                                        ex[:, 0:QC],
                                        start=(kt == 0), stop=(kt == NKB - 1),
                                    )
                                    nc.tensor.matmul(
                                        pv1[:],
                                        Vaug[:, vb + DH + 1:vb + VSTRIDE],
                                        ex[:, QC:2 * QC],
                                        start=(kt == 0), stop=(kt == NKB - 1),
                                    )
                                return f
                            pending[0] = mk_pv()
                            # late-b1 norm/oproj units go to the post-B tail:
                            # pumping them here makes the scheduler cram them
                            # into the last qc-boundary bubbles, jamming the
                            # PE queue ahead of the final sweeps
                            pump(allow_pc=not (b == 1 and qc >= 1))
                        pending[0]()
                        pending[0] = None
                        for h, pv in ((0, pv0), (1, pv1)):
                            a65 = apool.tile([65, QC], BF16, tag="a65")
                            nc.vector.tensor_copy(a65[:], pv[:])
                            nc.vector.tensor_copy(
                                attnU[b][h * 64:(h + 1) * 64, lqsl],
                                a65[0:64, :],
                            )
                            nc.vector.tensor_copy(
                                zb[b][32 * h:32 * h + 1, lqsl],
                                a65[64:65, :],
                            )
                        pc.append(("norm", b, qc))
                        for rc in range(qc * 4, qc * 4 + 4):
                            pc.append(("cu", b, rc))

                # drain whatever the slots didn't absorb (post-B: tail mode)
                with sidep():
                    while a1:
                        a1.pop(0)()
                    while pc:
                        run_pc(pc.pop(0), tail=True)

    nc.compile()
    _NC_CACHE["nc"] = nc
    return nc


def _shard_inputs(x, W_qkv, b_qkv, W_o):
    import ml_dtypes
    BF = ml_dtypes.bfloat16
    xT = np.ascontiguousarray(
        x.reshape(BL, D_MODEL).T.astype(BF)
    )
    ident = np.eye(128, dtype=BF)

    def lhsT_layout(w):
        # [D_MODEL, 128] -> [128, NKT*128] with [p, kt*128+ch] = w[kt*128+p, ch]
        return np.ascontiguousarray(
            w.reshape(NKT, 128, 128).transpose(1, 0, 2)
            .reshape(128, NKT * 128).astype(BF)
        )

    in_maps = []
    for c in range(NCORES):
        cs = slice(c * 128, (c + 1) * 128)
        wq = W_qkv[:, cs]
        wk = W_qkv[:, D_MODEL:][:, cs]
        wv = W_qkv[:, 2 * D_MODEL:][:, cs]
        in_maps.append({
            "xT": xT,
            "wq": lhsT_layout(wq), "wk": lhsT_layout(wk),
            "wv": lhsT_layout(wv),
            "bq": np.ascontiguousarray(
                b_qkv[cs], dtype=np.float32).reshape(128, 1),
            "bk": np.ascontiguousarray(
                b_qkv[D_MODEL:][cs], dtype=np.float32).reshape(128, 1),
            "bv": np.ascontiguousarray(
                b_qkv[2 * D_MODEL:][cs], dtype=np.float32).reshape(128, 1),
            "wo": np.ascontiguousarray(W_o[cs, :].astype(BF)),
            "ident": ident,
        })
    return in_maps


def _run(inputs, trace=False, tmpdir=None):
    from concourse.bass_utils import run_bass_kernel_spmd

    _register_ntff_hook()
    nc = _build()
    in_maps = _shard_inputs(
        np.asarray(inputs["x"], dtype=np.float32),
        np.asarray(inputs["W_qkv"], dtype=np.float32),
        np.asarray(inputs["b_qkv"], dtype=np.float32),
        np.asarray(inputs["W_o"], dtype=np.float32),
    )
    res = run_bass_kernel_spmd(nc, in_maps, core_ids=list(range(NCORES)),
                               trace=trace, tmpdir=tmpdir)
    partial = np.zeros((BL, D_MODEL), dtype=np.float64)
    for c in range(NCORES):
        partial += res.results[c]["out"].astype(np.float64)
    out = (partial + np.asarray(inputs["b_o"], dtype=np.float64)).astype(np.float32)
    return out.reshape(B, L, D_MODEL), res


def kernel(**inputs) -> np.ndarray:
    out, _ = _run(inputs, trace=False)
    return out


# revision 40
# speedup vs baseline: 1.0551x; 1.0010x over previous
"""Multi-head attention (b=2, l=2048, d_model=1024, h=16) on 8 trn2 NeuronCores.

Sharding: tensor-parallel over heads. Each core owns 2 heads: it computes the
QKV projections for its 128 channels (transposed layout), attention for its
heads, and a rank-128 partial of the output projection. The host sums the 8
partials and adds b_o (the tensor-parallel all-reduce, done at gather time).

v2 design (ACT-paced, PE row-tiled):
  The scalar engine (exp) is the theoretical floor: 16.8M exps/core at
  1 elem/lane/cycle @1.2GHz + 352cyc/op overhead ~= 147us. Everything else
  hides under it.
  warmup:  identity matmul burst (HAM clock ramp) + dummy exp (table preload).
  A0:      chunk-0 QKV projections only (~6us), so phase B starts early.
  B:       per (batch, 512-token q-chunk, k-tile): the two heads' scoresT
           matmuls (K=64 each) are issued back-to-back so the PE runs them
           CONCURRENTLY in row groups 0-63 / 64-127 (tile_position auto-derived
           from base_partition). One [128,1024] exp covers both heads. PV
           accumulates [65, 512] per head (ones-column computes Z). Emission is
           software-pipelined (sc(kt) -> exp(kt) -> pv(kt-1)) so ACT never
           waits. A pump queue fills PE slack with real work: remaining QKV
           chunks, V re-transposes, then normalize + output-projection units
           for finished q-chunks of both batches.
  norm:    selector matmul broadcasts Z over partitions; reciprocal_approx_fast
           (5x faster than reciprocal, plenty for softmax) + one multiply
           normalizes attnU in place, then the 2x[128,512] output projection,
           bf16 staging, DMA out.
"""
import sys
import types

import numpy as np

D_MODEL = 1024
H = 16
DH = 64
B = 2
L = 2048
BL = B * L            # 4096 tokens
NCORES = 8
NKT = D_MODEL // 128  # 8 feature tiles
TCH = 512             # phase-A token chunk
NCH = BL // TCH       # 8 chunks
QC = 512              # phase-B q chunk
NQC = L // QC         # 4 per batch
NKB = L // 128        # 16 k-tiles per batch
VSTRIDE = 2 * (DH + 1)  # per-k-tile Vaug columns: [V_h0 | 1 | V_h1 | 1]


def _register_ntff_hook():
    """Install the axon NTFF profiling hook module if the image lacks it.

    Harmless if never used; required for run_bass_kernel_spmd(trace=True)."""
    if "antenv.axon_hooks" in sys.modules:
        return
    try:
        import antenv
        mod = types.ModuleType("antenv.axon_hooks")
        holder = {}
        mod.set_axon_ntff_profile_hook = lambda h: holder.__setitem__("h", h)
        mod.get_axon_ntff_profile_hook = lambda: holder.get("h")
        sys.modules["antenv.axon_hooks"] = mod
        antenv.axon_hooks = mod
        from trn_agent_boot.trn_boot import _ntff_profile_via_ctypes
        mod.set_axon_ntff_profile_hook(
            _ntff_profile_via_ctypes("/opt/axon/libaxon_pjrt.so")
        )
    except Exception:
        pass


_NC_CACHE = {}


def _build():
    if "nc" in _NC_CACHE:
        return _NC_CACHE["nc"]
    import concourse.bacc as bacc
    import concourse.tile as tile
    import concourse.mybir as mybir

    F32 = mybir.dt.float32
    BF16 = mybir.dt.bfloat16
    AF = mybir.ActivationFunctionType
    ALU = mybir.AluOpType

    nc = bacc.Bacc("TRN2", target_bir_lowering=False, debug=False)

    xT_d = nc.dram_tensor("xT", [D_MODEL, BL], BF16, kind="ExternalInput").ap()
    wq_d = nc.dram_tensor("wq", [128, NKT * 128], BF16, kind="ExternalInput").ap()
    wk_d = nc.dram_tensor("wk", [128, NKT * 128], BF16, kind="ExternalInput").ap()
    wv_d = nc.dram_tensor("wv", [128, NKT * 128], BF16, kind="ExternalInput").ap()
    bq_d = nc.dram_tensor("bq", [128, 1], F32, kind="ExternalInput").ap()
    bk_d = nc.dram_tensor("bk", [128, 1], F32, kind="ExternalInput").ap()
    bv_d = nc.dram_tensor("bv", [128, 1], F32, kind="ExternalInput").ap()
    wo_d = nc.dram_tensor("wo", [128, D_MODEL], BF16, kind="ExternalInput").ap()
    id_d = nc.dram_tensor("ident", [128, 128], BF16, kind="ExternalInput").ap()
    out_d = nc.dram_tensor("out", [BL, D_MODEL], BF16, kind="ExternalOutput").ap()

    with tile.TileContext(nc) as tc:
        with (
            tc.tile_pool(name="weights", bufs=1) as wpool,
            tc.tile_pool(name="persist", bufs=1) as ppool,
        ):
            id_t = wpool.tile([128, 128], BF16, tag="ident")
            nc.gpsimd.dma_start(id_t[:], id_d)
            wq_t = wpool.tile([128, NKT * 128], BF16, tag="wq")
            wk_t = wpool.tile([128, NKT * 128], BF16, tag="wk")
            wv_t = wpool.tile([128, NKT * 128], BF16, tag="wv")
            bq_t = wpool.tile([128, 1], F32, tag="bq")
            bk_t = wpool.tile([128, 1], F32, tag="bk")
            bv_t = wpool.tile([128, 1], F32, tag="bv")
            wo_t = wpool.tile([128, D_MODEL], BF16, tag="wo")
            for t, d in ((wq_t, wq_d), (wk_t, wk_d), (wv_t, wv_d),
                         (bq_t, bq_d), (bk_t, bk_d), (bv_t, bv_d),
                         (wo_t, wo_d)):
                nc.gpsimd.dma_start(t[:], d)

            QT = ppool.tile([128, BL], BF16, tag="QT")
            KT = ppool.tile([128, BL], BF16, tag="KT")
            VT = ppool.tile([128, BL], BF16, tag="VT")
            Vaug = ppool.tile([128, (BL // 128) * VSTRIDE], BF16, tag="Vaug")
            attnU = [ppool.tile([128, L], BF16, tag=f"attnU{b}",
                                name=f"attnU{b}") for b in range(B)]
            # softmax denominators Z: h0 at partition 0, h1 at
            # partition 32 (engine writes need 32-aligned base partitions)
            zb = [ppool.tile([33, L], BF16, tag=f"zb{b}",
                             name=f"zb{b}") for b in range(B)]
            # head-half selector: rows 0 / 32 pick head halves, rest zero
            sel_t = ppool.tile([33, 128], BF16, tag="sel")
            scr = ppool.tile([1, 32], F32, tag="scr")

            # packed pair of bf16 1.0s viewed as f32
            ONE2 = float(np.frombuffer(
                np.uint32(0x3F803F80).tobytes(), dtype=np.float32)[0])
            nc.vector.memset(Vaug[:].bitcast(F32), ONE2)
            nc.vector.memset(sel_t[:].bitcast(F32), 0.0)
            nc.vector.memset(sel_t[:].bitcast(F32)[0:1, 0:32], ONE2)
            nc.vector.memset(sel_t[:].bitcast(F32)[32:33, 32:64], ONE2)
            for b in range(B):
                nc.vector.memset(zb[b][:].bitcast(F32), ONE2)

            with (
                tc.tile_pool(name="xin", bufs=3) as xpool,
                tc.tile_pool(name="scaleP", bufs=2) as spool,
                tc.tile_pool(name="expP", bufs=3) as epool,
                tc.tile_pool(name="a65P", bufs=2) as apool,
                tc.tile_pool(name="oout", bufs=3) as opool,
                tc.tile_pool(name="psX", bufs=2, space="PSUM") as psX,
                tc.tile_pool(name="psS", bufs=2, space="PSUM") as psS,
                tc.tile_pool(name="psPV", bufs=1, space="PSUM") as psPV,
            ):
                # ---- warmup: preload exp table + lift clock gate ----
                # wue is memset (no DMA dependency) so the warmup burst and
                # the ACT table load start immediately, overlapping the
                # weight/x DMAs.
                wue = ppool.tile([128, 128], BF16, tag="wue")
                nc.vector.memset(wue[:].bitcast(F32), ONE2)
                nc.scalar.activation(scr[:], wue[0:1, 0:64].bitcast(F32),
                                     AF.Exp)
                wu = psX.tile([128, 512], F32, tag="x")
                for i in range(40):
                    nc.tensor.matmul(wu[:, 0:128], wue[:], wue[:],
                                     start=(i == 0), stop=(i == 39))

                chunk_xt = {}

                def emit_chunk_dma(c):
                    xt = xpool.tile([128, NKT, TCH], BF16, tag="xchunk",
                                    name=f"xt{c}")
                    sl = slice(c * TCH, (c + 1) * TCH)
                    for kt in range(NKT):
                        nc.sync.dma_start(
                            xt[:, kt, :], xT_d[kt * 128:(kt + 1) * 128, sl]
                        )
                    chunk_xt[c] = xt

                proj_ps = {}

                def emit_proj_half(c, key, w_t, b_t, dst, half):
                    # one QKV projection = 8 accumulating matmuls, split in
                    # two 4-matmul halves sized to phase B's per-step PE
                    # slack. The two halves MUST be adjacent pump items:
                    # with psX bufs=2, one foreign allocation between them
                    # is safe, two would recycle the held buffer.
                    if half == 0:
                        ps = psX.tile([128, TCH], F32, tag="x",
                                      name=f"pj{key}{c}")
                        proj_ps[(key, c)] = ps
                    else:
                        ps = proj_ps.pop((key, c))
                    xt = chunk_xt[c]
                    for kt in range(half * 4, half * 4 + 4):
                        nc.tensor.matmul(
                            ps[:], w_t[:, kt * 128:(kt + 1) * 128],
                            xt[:, kt, :],
                            start=(kt == 0), stop=(kt == NKT - 1),
                        )
                    if half == 1:
                        sl = slice(c * TCH, (c + 1) * TCH)
                        nc.vector.tensor_scalar_add(dst[:, sl], ps[:],
                                                    b_t[:, 0:1])

                def emit_proj(c, key, w_t, b_t, dst):
                    emit_proj_half(c, key, w_t, b_t, dst, 0)
                    emit_proj_half(c, key, w_t, b_t, dst, 1)

                def emit_tr(c):
                    # natural-layout V (with ones cols) for this chunk's tiles
                    for g in range(c * (TCH // 128), (c + 1) * (TCH // 128)):
                        ps = psX.tile([128, 512], F32, tag="x", name="trps")
                        tp = ps.bitcast(BF16)
                        nc.tensor.transpose(
                            tp[:, 0:128], VT[:, g * 128:(g + 1) * 128], id_t[:]
                        )
                        base = g * VSTRIDE
                        nc.vector.tensor_copy(
                            Vaug[:, base:base + DH], tp[:, 0:DH]
                        )
                        nc.vector.tensor_copy(
                            Vaug[:, base + DH + 1:base + 2 * DH + 1],
                            tp[:, DH:2 * DH],
                        )

                def emit_norm_qc(b, qc):
                    # normalize 512 tokens: selector matmul broadcasts Z over
                    # the channel partitions, fast reciprocal in that layout,
                    # one multiply normalizes attnU in place
                    ps = psX.tile([128, 512], F32, tag="x", name="scaleps")
                    jsl = slice(qc * QC, (qc + 1) * QC)
                    nc.tensor.matmul(ps[:], sel_t[:], zb[b][:, jsl],
                                     start=True, stop=True)
                    ss = spool.tile([128, 512], F32, tag="ss", name="sstile")
                    nc.vector.reciprocal_approx_fast(ss[:], ps[:])
                    nc.vector.tensor_tensor(
                        attnU[b][:, jsl], attnU[b][:, jsl], ss[:],
                        op=ALU.mult,
                    )

                def emit_cu_rc(b, rc, tail=False):
                    # output projection for 128 tokens: out[tok, :] =
                    # attnN.T @ Wo, staged to bf16 and DMA'd out.
                    # In-B: matmul writes bf16 PSUM so the staging copy runs
                    # at the DVE's 2x 16-bit rate. Tail (post-B): the free
                    # scores banks hold both halves, one wide copy, ACT/DVE
                    # alternating.
                    jsl = slice(rc * 128, (rc + 1) * 128)
                    st = opool.tile([128, 1024], BF16, tag="cu", name="cust")
                    if tail:
                        ps = psS.tile([128, 2 * QC], F32, tag="sc",
                                      name="cutps")
                        for oc in range(2):
                            osl = slice(oc * 512, (oc + 1) * 512)
                            nc.tensor.matmul(ps[:, osl], attnU[b][:, jsl],
                                             wo_t[:, osl],
                                             start=True, stop=True)
                        if rc % 2 == 0:
                            nc.scalar.activation(st[:], ps[:], AF.Copy)
                        else:
                            nc.vector.tensor_copy(st[:], ps[:])
                    else:
                        for oc in range(2):
                            osl = slice(oc * 512, (oc + 1) * 512)
                            ps = psX.tile([128, 512], F32, tag="x",
                                          name="cups")
                            nc.tensor.matmul(ps[:], attnU[b][:, jsl],
                                             wo_t[:, osl],
                                             start=True, stop=True)
                            nc.vector.tensor_copy(st[:, osl], ps[:])
                    grow = b * 16 + rc
                    nc.sync.dma_start(
                        out_d[grow * 128:(grow + 1) * 128, :], st[:]
                    )

                # ---- A0: chunk 0 only, so phase B starts early ----
                # A0 carries ONLY what gates phase B's first step (K0, Q0):
                # anything else emitted critical-band before the B loop would
                # serialize ahead of it (lower priority + ready first).
                emit_chunk_dma(0)
                emit_chunk_dma(1)
                emit_proj(0, "k", wk_t, bk_t, KT)
                emit_proj(0, "q", wq_t, bq_t, QT)

                # ---- side-work queues for phase B's PE slack ----
                # NOTE: chunk c's Q-projection must be EMITTED before chunk
                # c+2's DMA (xpool bufs=2 reuses its buffer) so the WAR
                # dependency is tracked; keeping each chunk's group together
                # guarantees this.
                a1 = []

                def add_kv(c):
                    for key, w_t, b_t, dst in (("k", wk_t, bk_t, KT),
                                               ("v", wv_t, bv_t, VT)):
                        for h in range(2):
                            a1.append(lambda c=c, key=key, w_t=w_t, b_t=b_t,
                                      dst=dst, h=h:
                                      emit_proj_half(c, key, w_t, b_t,
                                                     dst, h))
                    a1.append(lambda c=c: emit_tr(c))

                def add_q(c):
                    for h in range(2):
                        a1.append(lambda c=c, h=h:
                                  emit_proj_half(c, "q", wq_t, bq_t, QT, h))

                def add_dma(c):
                    a1.append(lambda c=c: emit_chunk_dma(c))

                def add_v0(c=0):
                    for h in range(2):
                        a1.append(lambda h=h:
                                  emit_proj_half(0, "v", wv_t, bv_t, VT, h))
                    a1.append(lambda: emit_tr(0))

                # dma(c) reuses buffer c%3: it must follow chunk c-3's
                # reader emissions. V0/tr0 lead (PV kt0 pulls them in
                # immediately); each chunk's Q precedes its K/V so b0's
                # Q-projections land well before their qc sweeps.
                add_v0()
                add_dma(2)
                add_q(1)
                add_kv(1)
                add_dma(3)
                add_q(2)
                add_kv(2)
                add_dma(4)
                add_q(3)
                add_kv(3)
                add_dma(5)
                add_kv(4)
                add_q(4)
                add_dma(6)
                add_kv(5)
                add_q(5)
                add_dma(7)
                add_kv(6)
                add_q(6)
                add_kv(7)
                add_q(7)

                pc = []

                # Side-work runs in a high-value priority band: the Tile
                # scheduler then only slots it where the critical path
                # (scores -> exp -> PV) leaves the engines idle.
                import contextlib
                SIDE_BASE = 5_000_000
                side_ctr = [0]

                @contextlib.contextmanager
                def sidep():
                    save = tc.cur_priority
                    tc.cur_priority = SIDE_BASE + side_ctr[0]
                    try:
                        yield
                    finally:
                        side_ctr[0] = tc.cur_priority - SIDE_BASE
                        tc.cur_priority = save

                def run_pc(item, tail=False):
                    kind, b, i = item
                    if kind == "norm":
                        emit_norm_qc(b, i)
                    else:
                        emit_cu_rc(b, i, tail=tail)

                def pump(allow_pc=True):
                    with sidep():
                        for _ in range(3):
                            if a1:
                                a1.pop(0)()
                            elif pc and allow_pc:
                                run_pc(pc.pop(0))

                # ---- B: attention, ACT-paced ----
                pending = [None]
                for b in range(B):
                    for qc in range(NQC):
                        q0 = b * L + qc * QC
                        lqsl = slice(qc * QC, (qc + 1) * QC)
                        pv0 = psPV.tile([65, QC], F32, tag="pv0")
                        pv1 = psPV.tile([65, QC], F32, tag="pv1")
                        for kt in range(NKB):
                            ksl = slice(b * L + kt * 128,
                                        b * L + (kt + 1) * 128)
                            sc = psS.tile([128, 2 * QC], F32, tag="sc")
                            # two heads issued adjacently -> concurrent
                            # row-group matmuls (rows 0-63 / 64-127)
                            nc.tensor.matmul(sc[:, 0:QC], KT[0:64, ksl],
                                             QT[0:64, q0:q0 + QC],
                                             start=True, stop=True)
                            nc.tensor.matmul(sc[:, QC:2 * QC],
                                             KT[64:128, ksl],
                                             QT[64:128, q0:q0 + QC],
                                             start=True, stop=True)
                            ex = epool.tile([128, 2 * QC], BF16, tag="ex")
                            nc.scalar.activation(ex[:], sc[:], AF.Exp,
                                                 scale=0.125)
                            if pending[0] is not None:
                                pending[0]()
                            g = b * NKB + kt

                            def mk_pv(g=g, kt=kt, ex=ex, pv0=pv0, pv1=pv1):
                                vb = g * VSTRIDE

                                def f():
                                    nc.tensor.matmul(
                                        pv0[:], Vaug[:, vb:vb + DH + 1],
                                        ex[:, 0:QC],
                                        start=(kt == 0), stop=(kt == NKB - 1),
                                    )
                                    nc.tensor.matmul(
                                        pv1[:],
                                        Vaug[:, vb + DH + 1:vb + VSTRIDE],
                                        ex[:, QC:2 * QC],
                                        start=(kt == 0), stop=(kt == NKB - 1),
                                    )
                                return f
                            pending[0] = mk_pv()
                            # late-b1 norm/oproj units go to the post-B tail:
                            # pumping them here makes the scheduler cram them
                            # into the last qc-boundary bubbles, jamming the
                            # PE queue ahead of the final sweeps
                            pump(allow_pc=not (b == 1 and qc >= 1))
                        pending[0]()
                        pending[0] = None
                        for h, pv in ((0, pv0), (1, pv1)):
                            a65 = apool.tile([65, QC], BF16, tag="a65")
                            nc.vector.tensor_copy(a65[:], pv[:])
                            nc.vector.tensor_copy(
                                attnU[b][h * 64:(h + 1) * 64, lqsl],
                                a65[0:64, :],
                            )
                            nc.vector.tensor_copy(
                                zb[b][32 * h:32 * h + 1, lqsl],
                                a65[64:65, :],
                            )
                        pc.append(("norm", b, qc))
                        for rc in range(qc * 4, qc * 4 + 4):
                            pc.append(("cu", b, rc))

                # drain whatever the slots didn't absorb (post-B: tail mode)
                with sidep():
                    while a1:
                        a1.pop(0)()
                    while pc:
                        run_pc(pc.pop(0), tail=True)

    nc.compile()
    _NC_CACHE["nc"] = nc
    return nc


def _shard_inputs(x, W_qkv, b_qkv, W_o):
    import ml_dtypes
    BF = ml_dtypes.bfloat16
    xT = np.ascontiguousarray(
        x.reshape(BL, D_MODEL).T.astype(BF)
    )
    ident = np.eye(128, dtype=BF)

    def lhsT_layout(w):
        # [D_MODEL, 128] -> [128, NKT*128] with [p, kt*128+ch] = w[kt*128+p, ch]
        return np.ascontiguousarray(
            w.reshape(NKT, 128, 128).transpose(1, 0, 2)
            .reshape(128, NKT * 128).astype(BF)
        )

    in_maps = []
    for c in range(NCORES):
        cs = slice(c * 128, (c + 1) * 128)
        wq = W_qkv[:, cs]
        wk = W_qkv[:, D_MODEL:][:, cs]
        wv = W_qkv[:, 2 * D_MODEL:][:, cs]
        in_maps.append({
            "xT": xT,
            "wq": lhsT_layout(wq), "wk": lhsT_layout(wk),
            "wv": lhsT_layout(wv),
            "bq": np.ascontiguousarray(
                b_qkv[cs], dtype=np.float32).reshape(128, 1),
            "bk": np.ascontiguousarray(
                b_qkv[D_MODEL:][cs], dtype=np.float32).reshape(128, 1),
            "bv": np.ascontiguousarray(
                b_qkv[2 * D_MODEL:][cs], dtype=np.float32).reshape(128, 1),
            "wo": np.ascontiguousarray(W_o[cs, :].astype(BF)),
            "ident": ident,
        })
    return in_maps


def _run(inputs, trace=False, tmpdir=None):
    from concourse.bass_utils import run_bass_kernel_spmd

    _register_ntff_hook()
    nc = _build()
    in_maps = _shard_inputs(
        np.asarray(inputs["x"], dtype=np.float32),
        np.asarray(inputs["W_qkv"], dtype=np.float32),
        np.asarray(inputs["b_qkv"], dtype=np.float32),
        np.asarray(inputs["W_o"], dtype=np.float32),
    )
    res = run_bass_kernel_spmd(nc, in_maps, core_ids=list(range(NCORES)),
                               trace=trace, tmpdir=tmpdir)
    partial = np.zeros((BL, D_MODEL), dtype=np.float64)
    for c in range(NCORES):
        partial += res.results[c]["out"].astype(np.float64)
    out = (partial + np.asarray(inputs["b_o"], dtype=np.float64)).astype(np.float32)
    return out.reshape(B, L, D_MODEL), res


def kernel(**inputs) -> np.ndarray:
    out, _ = _run(inputs, trace=False)
    return out
